# revision 9
# baseline (speedup 1.0000x reference)
"""GAT (2-layer, 4-head) Trainium2 Bass kernel, SPMD over 8 NeuronCores.

Sharding: 1D row partition of N. Each core computes its N/8-row block of
both attention layers. h (per-head projected features) is computed
replicated on every core (cheap); the layer-2 input features are exchanged
with a single AllGather.

Layout trick: scores are computed directly in TRANSPOSED form p[j, i]
(j = source node on partitions, i = destination rows of this core's block
on the free axis), so that
  - softmax denominators come free from the aggregation matmul via a
    ones-column prepended to the feature matrix (flash-attention style,
    unnormalized exp then one divide at the end), and
  - no on-device transposes of the N x N score matrix are ever needed.
The adjacency mask and X are fed pre-transposed from the host.
"""

import numpy as np
import ml_dtypes
from contextlib import ExitStack

BF16 = ml_dtypes.bfloat16

# problem constants (hardcoded per contract)
N, FIN, FH, H, FO = 4096, 512, 128, 4, 64
N_CORES = 8
FCAT = H * FH  # 512
GRW = 2 + FO   # gather row width: [s2_o | ones-slot | h2 features]


def _cfg(n=N, n_cores=N_CORES):
    nb = n // n_cores        # rows per core
    return dict(
        n=n, nb=nb, n_cores=n_cores,
        jc=n // 128,          # j (source-node) chunks of 128
        kc=FIN // 128,        # contraction chunks for X @ W
        ic=(nb + 127) // 128, # i chunks for layer-2 natural-out aggregation
    )


def host_prep(X, adjacency_matrix, W_h, b_h, a1_h, a2_h, ab_h,
              W_o, b_o, a1_o, a2_o, ab_o, cfg):
    """Fold weights / biases on the host; produce per-core input maps."""
    f32 = np.float32
    X = np.asarray(X, f32)
    adj = np.asarray(adjacency_matrix)
    W_h, b_h = np.asarray(W_h, f32), np.asarray(b_h, f32)
    a1_h, a2_h, ab_h = np.asarray(a1_h, f32), np.asarray(a2_h, f32), np.asarray(ab_h, f32)
    W_o, b_o = np.asarray(W_o, f32), np.asarray(b_o, f32)
    a1_o, a2_o, ab_o = np.asarray(a1_o, f32), np.asarray(a2_o, f32), np.asarray(ab_o, f32)

    nb, ncores = cfg["nb"], cfg["n_cores"]

    XT = np.ascontiguousarray(X.T).astype(BF16)                  # [FIN, n]
    maskT = np.ascontiguousarray((adj > 0).T.astype(f32)).astype(BF16)

    WC = np.concatenate([W_h[h] for h in range(H)], axis=1).astype(BF16)
    wa1 = np.stack([W_h[h] @ a1_h[h] for h in range(H)], axis=1)  # [FIN, H]
    wa2 = np.stack([W_h[h] @ a2_h[h] for h in range(H)], axis=1)  # [FIN, H]
    WA4 = wa2.astype(BF16)
    WA8 = np.concatenate([wa1, wa2], axis=1).astype(BF16)         # [FIN, 2H]
    c1 = np.array([b_h[h] @ a1_h[h] + ab_h[h] for h in range(H)], f32)
    c2 = np.array([b_h[h] @ a2_h[h] for h in range(H)], f32)
    c12 = (c1 + c2).astype(f32)

    wao1 = W_o @ a1_o
    wao2 = W_o @ a2_o
    # elu(-1) fold: device computes x_raw = elu(z)+1; corrections:
    c1o = float(b_o @ a1_o + ab_o - wao1.sum())
    c2o = float(b_o @ a2_o - wao2.sum())
    beta = (b_o - W_o.sum(axis=0)).astype(f32)                    # [FO]
    BETA = np.ascontiguousarray(np.broadcast_to(beta, (128, FO)))

    GR = np.concatenate(
        [wao2[:, None], np.zeros((FCAT, 1), f32), W_o], axis=1
    ).astype(BF16)                                                # [FCAT, GRW]
    WAO1 = wao1[:, None].astype(BF16)
    BH = np.stack([b_h[h] for h in range(H)], axis=1).astype(f32) # [FH, H]

    shared = {
        "XT": XT, "WC": WC, "WA4": WA4, "WA8": WA8,
        "GR": GR, "WAO1": WAO1, "BH": BH, "BETA": BETA,
    }
    in_maps = []
    for c in range(ncores):
        m = dict(shared)
        m["XTo"] = np.ascontiguousarray(XT[:, c * nb:(c + 1) * nb])
        m["MT"] = np.ascontiguousarray(maskT[:, c * nb:(c + 1) * nb])
        in_maps.append(m)
    return in_maps, {"c12": c12, "c12o": c1o + c2o}


def build_program(nc, tc, cfg, consts, skip_collective=False):
    """Emit the SPMD GAT program into TileContext tc."""
    from concourse import mybir

    f32 = mybir.dt.float32
    bf16 = mybir.dt.bfloat16
    AF = mybir.ActivationFunctionType
    OP = mybir.AluOpType
    AX = mybir.AxisListType

    n, nb, jcn, kcn, icn = cfg["n"], cfg["nb"], cfg["jc"], cfg["kc"], cfg["ic"]
    ncores = cfg["n_cores"]
    c12, c12o = consts["c12"], consts["c12o"]
    FH2 = FH // 2
    HCW = FH + 1  # 129: [ones | 128 features] per (head, jc)

    # ---- DRAM I/O ----
    d_XT = nc.dram_tensor("XT", [FIN, n], bf16, kind="ExternalInput")
    d_XTo = nc.dram_tensor("XTo", [FIN, nb], bf16, kind="ExternalInput")
    d_MT = nc.dram_tensor("MT", [n, nb], bf16, kind="ExternalInput")
    d_WC = nc.dram_tensor("WC", [FIN, FCAT], bf16, kind="ExternalInput")
    d_WA4 = nc.dram_tensor("WA4", [FIN, H], bf16, kind="ExternalInput")
    d_WA8 = nc.dram_tensor("WA8", [FIN, 2 * H], bf16, kind="ExternalInput")
    d_GR = nc.dram_tensor("GR", [FCAT, GRW], bf16, kind="ExternalInput")
    d_WAO1 = nc.dram_tensor("WAO1", [FCAT, 1], bf16, kind="ExternalInput")
    d_BH = nc.dram_tensor("BH", [FH, H], f32, kind="ExternalInput")
    d_BETA = nc.dram_tensor("BETA", [128, FO], f32, kind="ExternalInput")
    d_OUT = nc.dram_tensor("OUT", [nb, FO], f32, kind="ExternalOutput")

    ctx = ExitStack()
    with ctx:
        cpool = ctx.enter_context(tc.tile_pool(name="const", bufs=1))
        work = ctx.enter_context(tc.tile_pool(name="work", bufs=3))
        spool = ctx.enter_context(tc.tile_pool(name="small", bufs=2))
        dpool = ctx.enter_context(tc.tile_pool(name="dram", bufs=1, space="DRAM"))

        def load(name, dram, parts, width, dt=bf16, rearr=True):
            t = cpool.tile([parts, width], dt, tag=name)
            if rearr:
                src = dram.ap().rearrange("(c p) x -> p c x", p=parts)
                dst = t[:].rearrange("p (c x) -> p c x", c=src.shape[1])
                nc.sync.dma_start(dst, src)
            else:
                nc.sync.dma_start(t[:], dram.ap())
            return t

        XT_sb = load("XT", d_XT, 128, kcn * n)
        XTo_sb = load("XTo", d_XTo, 128, kcn * nb)
        MT_sb = load("MT", d_MT, 128, jcn * nb)
        WC_sb = load("WC", d_WC, 128, kcn * FCAT)
        WA4_sb = load("WA4", d_WA4, 128, kcn * H)
        WA8_sb = load("WA8", d_WA8, 128, kcn * 2 * H)
        GR_sb = load("GR", d_GR, 128, kcn * GRW)
        WAO1_sb = load("WAO1", d_WAO1, 128, kcn)
        BH_sb = load("BH", d_BH, FH, H, dt=f32, rearr=False)
        BETA_sb = load("BETA", d_BETA, 128, FO, dt=f32, rearr=False)

        onesb = cpool.tile([1, 128], bf16, tag="ones")
        nc.vector.memset(onesb[:], 1.0)

        h_all = cpool.tile([128, H * jcn * HCW], bf16, tag="h_all")
        nc.vector.memset(h_all[:, FH2:H * jcn * HCW:HCW], 1.0)  # ones columns

        s2colT = cpool.tile([128, jcn * H], f32, tag="s2colT")
        s1rows = cpool.tile([1, H * nb], bf16, tag="s1rows")
        xT_sb = cpool.tile([128, kcn * nb], bf16, tag="xT")
        h2g_sb = cpool.tile([128, jcn * GRW], bf16, tag="h2g")
        s2oT = cpool.tile([128, jcn], f32, tag="s2oT")

        cbias = cpool.tile([128, H + 1], f32, tag="cbias")
        for hh in range(H):
            nc.vector.memset(cbias[:, hh:hh + 1], float(c12[hh]))
        nc.vector.memset(cbias[:, H:H + 1], float(c12o))

        # ---- Phase 1: h-build (replicated) + fused s2 columns ----
        with tc.tile_pool(name="pp_a", bufs=2, space="PSUM") as pp_a, \
             tc.tile_pool(name="pp_b", bufs=1, space="PSUM") as pp_b:
            for jc in range(jcn):
                ph = pp_a.tile([128, FCAT], f32, tag="hb")
                ps = pp_b.tile([128, H], f32, tag="sf")
                for kc in range(kcn):
                    lhs = XT_sb[:, kc * n + jc * 128: kc * n + jc * 128 + 128]
                    nc.tensor.matmul(ph[:], lhs, WC_sb[:, kc * FCAT:(kc + 1) * FCAT],
                                     start=(kc == 0), stop=(kc == kcn - 1))
                    nc.tensor.matmul(ps[:], lhs, WA4_sb[:, kc * H:(kc + 1) * H],
                                     start=(kc == 0), stop=(kc == kcn - 1))
                hv = h_all[:].rearrange("p (h jc w) -> p h jc w", h=H, jc=jcn)
                pv = ph[:].rearrange("p (h f) -> p h f", h=H)
                nc.vector.tensor_copy(hv[:, :, jc, 0:FH2], pv[:, :, 0:FH2])
                nc.vector.tensor_copy(hv[:, :, jc, FH2 + 1:HCW], pv[:, :, FH2:FH])
                sv = s2colT[:].rearrange("p (jc h) -> p jc h", jc=jcn)
                nc.vector.tensor_copy(sv[:, jc, :], ps[:])

            # ---- Phase 2: own-block s1 rows (one M=1 matmul per head) ----
            for hh in range(H):
                ps1 = pp_b.tile([1, nb], f32, tag="s1own", name=f"ps1_{hh}")
                for kc in range(kcn):
                    nc.tensor.matmul(ps1[:],
                                     WA8_sb[:, kc * 2 * H + hh: kc * 2 * H + hh + 1],
                                     XTo_sb[:, kc * nb:(kc + 1) * nb],
                                     start=(kc == 0), stop=(kc == kcn - 1))
                nc.scalar.copy(s1rows[:, hh * nb:(hh + 1) * nb], ps1[:])

        # ---- Phase 3: per-head layer-1 attention ----
        with tc.tile_pool(name="pp_c", bufs=1, space="PSUM") as pp_c:
            for h in range(H):
                psb = pp_c.tile([128, nb], f32, tag="s1b")
                nc.tensor.matmul(psb[:], onesb[:], s1rows[:, h * nb:(h + 1) * nb])
                S1b = spool.tile([128, nb], bf16, tag="S1b")
                nc.scalar.activation(S1b[:], psb[:], AF.Identity, bias=cbias[:, h:h + 1])

                pA = pp_c.tile([128, nb], f32, tag="aggA")
                pB = pp_c.tile([128, nb], f32, tag="aggB")
                for jc in range(jcn):
                    s2 = s2colT[:, jc * H + h: jc * H + h + 1]
                    a = work.tile([128, nb], bf16, tag="a")
                    nc.vector.tensor_scalar_add(a[:], S1b[:], s2)
                    b = work.tile([128, nb], bf16, tag="b")
                    nc.vector.tensor_scalar_mul(b[:], a[:], 0.2)
                    cm = work.tile([128, nb], bf16, tag="c")
                    nc.vector.tensor_tensor(cm[:], a[:], b[:], OP.max)
                    d = work.tile([128, nb], bf16, tag="d")
                    nc.scalar.activation(d[:], cm[:], AF.Exp)
                    p = work.tile([128, nb], bf16, tag="p")
                    nc.gpsimd.tensor_tensor(p[:], d[:],
                                            MT_sb[:, jc * nb:(jc + 1) * nb], OP.mult)
                    base = (h * jcn + jc) * HCW
                    nc.tensor.matmul(pA[0:FH2 + 1, :], h_all[:, base:base + FH2 + 1],
                                     p[:], start=(jc == 0), stop=(jc == jcn - 1))
                    nc.tensor.matmul(pB[0:FH2, :],
                                     h_all[:, base + FH2 + 1:base + HCW],
                                     p[:], start=(jc == 0), stop=(jc == jcn - 1))

                # head finalize: 1/denom via exp(-ln), broadcast, +b_h, elu(+1)
                lnd = spool.tile([1, nb], f32, tag="lnd")
                nc.scalar.activation(lnd[:], pA[FH2:FH2 + 1, :], AF.Ln)
                rec = spool.tile([1, nb], bf16, tag="rec")
                nc.scalar.activation(rec[:], lnd[:], AF.Exp, scale=-1.0)
                pR = pp_c.tile([128, nb], f32, tag="R")
                nc.tensor.matmul(pR[:], onesb[:], rec[:])

                gA = spool.tile([FH2, nb], bf16, tag="gA")
                nc.scalar.copy(gA[:], pA[0:FH2, :])
                gB = spool.tile([FH2, nb], bf16, tag="gB")
                nc.scalar.copy(gB[:], pB[0:FH2, :])

                v = spool.tile([128, nb], bf16, tag="v")
                nc.vector.tensor_tensor(v[0:FH2, :], gA[:], pR[0:FH2, :], OP.mult)
                nc.vector.tensor_tensor(v[FH2:FH, :], gB[:], pR[FH2:FH, :], OP.mult)
                v2 = spool.tile([128, nb], bf16, tag="v2")
                nc.vector.tensor_scalar_add(v2[:], v[:], BH_sb[:, h:h + 1])
                r = spool.tile([128, nb], bf16, tag="r")
                nc.vector.tensor_scalar_max(r[:], v2[:], 0.0)
                mn = spool.tile([128, nb], bf16, tag="mn")
                nc.vector.tensor_scalar_min(mn[:], v2[:], 0.0)
                E = spool.tile([128, nb], bf16, tag="E")
                nc.scalar.activation(E[:], mn[:], AF.Exp)
                nc.vector.tensor_tensor(xT_sb[:, h * nb:(h + 1) * nb], r[:], E[:],
                                        OP.add)

        # ---- Phase 4: gather input = [x_raw . wao2 | 0 | x_raw @ Wo] ----
        gin = dpool.tile([nb, GRW], bf16, tag="gin")
        gout = dpool.tile([n, GRW], bf16, tag="gout")
        with tc.tile_pool(name="pp_d", bufs=1, space="PSUM") as pp_d:
            for ic in range(icn):
                iw = min(128, nb - ic * 128)
                pg = pp_d.tile([128, GRW], f32, tag="g")
                for kc in range(kcn):
                    nc.tensor.matmul(
                        pg[0:iw, :],
                        xT_sb[:, kc * nb + ic * 128: kc * nb + ic * 128 + iw],
                        GR_sb[:, kc * GRW:(kc + 1) * GRW],
                        start=(kc == 0), stop=(kc == kcn - 1))
                gsb = spool.tile([128, GRW], bf16, tag="gsb")
                nc.scalar.copy(gsb[0:iw, :], pg[0:iw, :])
                nc.sync.dma_start(gin[ic * 128: ic * 128 + iw, :], gsb[0:iw, :])

            ps1o = pp_d.tile([1, nb], f32, tag="s1o")
            for kc in range(kcn):
                nc.tensor.matmul(ps1o[:], WAO1_sb[:, kc:kc + 1],
                                 xT_sb[:, kc * nb:(kc + 1) * nb],
                                 start=(kc == 0), stop=(kc == kcn - 1))
            s1orow = spool.tile([1, nb], bf16, tag="s1orow")
            nc.scalar.copy(s1orow[:], ps1o[:])
            psbo = pp_d.tile([128, nb], f32, tag="s1bo")
            nc.tensor.matmul(psbo[:], onesb[:], s1orow[:])
            S1bo = spool.tile([128, nb], bf16, tag="S1bo")
            nc.scalar.activation(S1bo[:], psbo[:], AF.Identity, bias=cbias[:, H:H + 1])

            # ---- AllGather ----
            if skip_collective:
                for cc_ in range(ncores):
                    nc.gpsimd.dma_start(gout[cc_ * nb:(cc_ + 1) * nb, :], gin[:])
            else:
                nc.gpsimd.collective_compute(
                    "AllGather", OP.bypass,
                    replica_groups=[list(range(ncores))],
                    ins=[gin[:].opt()], outs=[gout[:].opt()],
                )
            nc.sync.dma_start(h2g_sb[:].rearrange("p (jc x) -> p jc x", jc=jcn),
                              gout[:].rearrange("(jc p) x -> p jc x", p=128))
            nc.vector.memset(h2g_sb[:, 1:jcn * GRW:GRW], 1.0)  # ones col
            nc.vector.tensor_copy(
                s2oT[:],
                h2g_sb[:].rearrange("p (jc w) -> p jc w", jc=jcn)[:, :, 0])

        # ---- Phase 5: layer-2 attention (natural-out aggregation) ----
        with tc.tile_pool(name="pp_e", bufs=1, space="PSUM") as pp_e:
            vps = []
            for ic in range(icn):
                vt_ = pp_e.tile([128, 1 + FO], f32, tag=f"v{ic}")
                vps.append(vt_)
            for jc in range(jcn):
                a = work.tile([128, nb], bf16, tag="a")
                nc.vector.tensor_scalar_add(a[:], S1bo[:], s2oT[:, jc:jc + 1])
                b = work.tile([128, nb], bf16, tag="b")
                nc.vector.tensor_scalar_mul(b[:], a[:], 0.2)
                cm = work.tile([128, nb], bf16, tag="c")
                nc.vector.tensor_tensor(cm[:], a[:], b[:], OP.max)
                d = work.tile([128, nb], bf16, tag="d")
                nc.scalar.activation(d[:], cm[:], AF.Exp)
                p2 = work.tile([128, nb], bf16, tag="p")
                nc.gpsimd.tensor_tensor(p2[:], d[:],
                                        MT_sb[:, jc * nb:(jc + 1) * nb], OP.mult)
                for ic in range(icn):
                    iw = min(128, nb - ic * 128)
                    nc.tensor.matmul(
                        vps[ic][0:iw, :], p2[:, ic * 128: ic * 128 + iw],
                        h2g_sb[:, jc * GRW + 1:(jc + 1) * GRW],
                        start=(jc == 0), stop=(jc == jcn - 1))

            # ---- Phase 6: normalize, +beta, elu(+1), log_softmax, store ----
            for ic in range(icn):
                iw = min(128, nb - ic * 128)
                vp = vps[ic]
                rc = spool.tile([128, 1], f32, tag="rc")
                nc.vector.reciprocal(rc[0:iw, :], vp[0:iw, 0:1])
                vv = spool.tile([128, FO], f32, tag="vv")
                nc.vector.tensor_scalar_mul(vv[0:iw, :], vp[0:iw, 1:1 + FO],
                                            rc[0:iw, :])
                vt = spool.tile([128, FO], f32, tag="vt")
                nc.vector.tensor_tensor(vt[0:iw, :], vv[0:iw, :], BETA_sb[0:iw, :],
                                        OP.add)
                r = spool.tile([128, FO], f32, tag="r2")
                nc.vector.tensor_scalar_max(r[0:iw, :], vt[0:iw, :], 0.0)
                mn = spool.tile([128, FO], f32, tag="mn2")
                nc.vector.tensor_scalar_min(mn[0:iw, :], vt[0:iw, :], 0.0)
                E = spool.tile([128, FO], f32, tag="E2")
                nc.scalar.activation(E[0:iw, :], mn[0:iw, :], AF.Exp)
                u = spool.tile([128, FO], f32, tag="u")
                nc.vector.tensor_tensor(u[0:iw, :], r[0:iw, :], E[0:iw, :], OP.add)
                nm = spool.tile([128, 1], f32, tag="nm")
                nc.vector.tensor_reduce(nm[0:iw, :], u[0:iw, :], AX.X, OP.max,
                                        negate=True)
                eu = spool.tile([128, FO], f32, tag="eu")
                se = spool.tile([128, 1], f32, tag="se")
                nc.scalar.activation(eu[0:iw, :], u[0:iw, :], AF.Exp,
                                     bias=nm[0:iw, :], accum_out=se[0:iw, :])
                L = spool.tile([128, 1], f32, tag="L")
                nc.scalar.activation(L[0:iw, :], se[0:iw, :], AF.Ln)
                cc = spool.tile([128, 1], f32, tag="cc")
                nc.vector.tensor_tensor(cc[0:iw, :], nm[0:iw, :], L[0:iw, :],
                                        OP.subtract)
                outf = spool.tile([128, FO], f32, tag="outf")
                nc.vector.tensor_scalar_add(outf[0:iw, :], u[0:iw, :], cc[0:iw, :])
                nc.sync.dma_start(d_OUT.ap()[ic * 128: ic * 128 + iw, :],
                                  outf[0:iw, :])

    return d_OUT


def run_gat(inputs, cfg=None, trace=False):
    import concourse.bacc as bacc
    import concourse.tile as tile
    from concourse.bass_utils import run_bass_kernel_spmd

    cfg = cfg or _cfg()
    in_maps, consts = host_prep(cfg=cfg, **inputs)

    nc = bacc.Bacc("TRN2", target_bir_lowering=False, debug=False,
                   num_devices=cfg["n_cores"])
    with tile.TileContext(nc) as tc:
        build_program(nc, tc, cfg, consts)
    nc.compile()

    res = run_bass_kernel_spmd(nc, in_maps, list(range(cfg["n_cores"])),
                               trace=trace)
    out = np.concatenate([res.results[c]["OUT"] for c in range(cfg["n_cores"])],
                         axis=0)
    return out.astype(np.float32), res


def kernel(**inputs) -> np.ndarray:
    out, _ = run_gat(inputs)
    return out
